# revision 20
# baseline (speedup 1.0000x reference)
"""TRN2 Bass kernel for nn_AVNNType1Linear.

Reference computation (B=2048, D_in=D_out=4096):
    act, carry = x[..., 0], x[..., 1]                  # x: [B, D_in, 2] f32
    act_out    = relu(act @ W.T + b)                   # [B, D_out]
    val        = 0.5*(mean(act, -1) + mean(carry, -1)) # [B]
    out        = stack([act_out, broadcast(val)], -1)  # [B, D_out, 2]

Distribution over 8 NeuronCores: 4-way data-parallel over batch x 2-way
tensor-parallel over output features (minimizes per-core HBM traffic:
xaT 8.4MB + xc 8.4MB + W.T-half 33.6MB + out 8.4MB ~= 59MB/core).

Per-core kernel: single-pass float32r matmul (full PE rate, ~2**-13
relative rounding vs ~2**-8 for bf16). The activator comes in host-
transposed ([D_in, B_loc]) so contraction sits on the partition dim with
clean DMAs; bias is folded into PSUM via a K=1 ones-row matmul; the
activator row-sums fall out of N=1 ones-column matmuls accumulated in
PSUM alongside the main GEMM; carry row-sums are a DVE free-dim reduce
of the naturally-laid-out carry channel. The [b, (o,ch)] interleaved
output tile is assembled in SBUF so the store DMA writes contiguous 4KB
rows.
"""

import os

import numpy as np

import concourse.mybir as mybir
import concourse.tile as tile
from concourse import bacc
from concourse.bass_utils import run_bass_kernel_spmd

B, D = 2048, 4096          # batch, D_in == D_out
M_SHARDS, F_SHARDS = 4, 2  # batch x feature grid over 8 cores
B_LOC = B // M_SHARDS      # 512 batch rows per core
O_LOC = D // F_SHARDS      # 2048 output features per core
KT = D // 128              # 32 contraction tiles
OT = O_LOC // 512          # 4 output tiles of 512
JT = B_LOC // 128          # 4 batch tiles of 128
KG = 8                     # activator SBUF tiles (groups of KT//KG k-tiles)
KPG = KT // KG


MM_DTYPE = os.environ.get("MM_DTYPE", "float16")


def _build():
    dt = mybir.dt
    mmdt = getattr(dt, MM_DTYPE)
    nc = bacc.Bacc("TRN2", target_bir_lowering=False, debug=False)
    xaT = nc.dram_tensor("xaT", [D, B_LOC], mmdt, kind="ExternalInput").ap()
    xn = nc.dram_tensor("xn", [B_LOC, 2 * D], mmdt, kind="ExternalInput").ap()
    wT = nc.dram_tensor("wT", [D, O_LOC], mmdt, kind="ExternalInput").ap()
    bias = nc.dram_tensor("bias", [1, O_LOC], mmdt, kind="ExternalInput").ap()
    ones = nc.dram_tensor("ones", [128, 128], mmdt, kind="ExternalInput").ap()
    out = nc.dram_tensor(
        "out", [B_LOC, O_LOC, 2], dt.float32, kind="ExternalOutput"
    ).ap()

    with tile.TileContext(nc) as tc:
        with (
            tc.tile_pool(name="persist", bufs=1) as persist,
            tc.tile_pool(name="wstream", bufs=16) as wpool,
            tc.tile_pool(name="xcpool", bufs=4) as xcpool,
            tc.tile_pool(name="opool", bufs=4) as opool,
            tc.tile_pool(name="small", bufs=1) as small,
            tc.tile_pool(name="ps", bufs=8, space="PSUM") as pspool,
        ):
            # --- persistent tiles (loads deferred off the critical path) --
            ones_sb = persist.tile([128, 128], mmdt)
            bias_sb = persist.tile([1, O_LOC], mmdt)

            def load_consts():
                nc.sync.dma_start(out=ones_sb, in_=ones)
                nc.sync.dma_start(out=bias_sb, in_=bias)

            # whole activator shard, [i%128, kt, b] layout, in KG chunks.
            # Only group 0 loads up front; later groups are emitted inside
            # the first k-loop so the w stream isn't queued behind them.
            act_g = [
                persist.tile([128, KPG, B_LOC], mmdt, tag=f"act{g}", name=f"act{g}")
                for g in range(KG)
            ]

            def load_act(g):
                nc.sync.dma_start(
                    out=act_g[g],
                    in_=xaT[g * KPG * 128 : (g + 1) * KPG * 128, :].rearrange(
                        "(kt p) b -> p kt b", p=128
                    ),
                )

            load_act(0)

            def act_tile(k, j):
                return act_g[k // KPG][:, k % KPG, j * 128 : (j + 1) * 128]

            # per-row total sums (both channels) -> val; the xn DMAs are
            # spread through the o==0 k-loop so they share bandwidth with
            # the w stream instead of ever queuing ahead of it
            csum_sb = small.tile([128, JT], dt.float32)
            val_sb = small.tile([128, JT], dt.float32)

            def row_sums(j):
                xn_t = xcpool.tile([128, 2 * D], mmdt, tag="xn", name=f"xn_{j}")
                nc.sync.dma_start(out=xn_t, in_=xn[j * 128 : (j + 1) * 128, :])
                nc.vector.reduce_sum(
                    csum_sb[:, j : j + 1], xn_t, axis=mybir.AxisListType.X
                )
            # o-pairs: all 8 PSUM banks live so each stationary act tile
            # feeds 2 consecutive matmuls (the weight-load cost amortizes).
            # Pair 0 runs k-outer (row sums/val finish early for its
            # epilogues); pair 1 runs j-outer over resident w tiles so the
            # epilogue + store of each batch tile streams out while later
            # batch tiles still compute (short kernel tail).

            def epilogue(ps_t, j, o):
                o_sl = slice(o * 512, (o + 1) * 512)
                out_t = opool.tile(
                    [128, 512, 2], dt.float32, tag="out", name=f"out_{o}_{j}"
                )
                nc.vector.tensor_scalar_max(out_t[:, :, 0], ps_t, 0.0)
                nc.vector.tensor_scalar(
                    out_t[:, :, 1], ps_t, 0.0, val_sb[:, j : j + 1],
                    op0=mybir.AluOpType.mult, op1=mybir.AluOpType.add,
                )
                eng = nc.sync if (j + o) % 2 == 0 else nc.gpsimd
                eng.dma_start(out=out[j * 128 : (j + 1) * 128, o_sl, :], in_=out_t)

            def w_pair_tile(kp, o0):
                w_t = wpool.tile(
                    [128, 2, 2, 512], mmdt, tag="wt", name=f"wt_{o0}_{kp}"
                )
                nc.sync.dma_start(
                    out=w_t,
                    in_=wT[
                        kp * 256 : (kp + 1) * 256, o0 * 512 : (o0 + 2) * 512
                    ].rearrange("(kt p) (ot n) -> p kt ot n", p=128, n=512),
                )
                return w_t

            # ---- pair 0: k-outer ----
            ps = [
                [
                    pspool.tile([128, 512], dt.float32, tag="ps", name=f"ps0_{ot}_{j}")
                    for j in range(JT)
                ]
                for ot in range(2)
            ]
            for kp in range(KT // 2):
                if 1 <= kp <= KG - 1:
                    load_act(kp)
                if kp == 1:
                    load_consts()
                if kp in (5, 7, 9, 11):
                    row_sums((kp - 5) // 2)
                w_t = w_pair_tile(kp, 0)
                for kk in range(2):
                    k = 2 * kp + kk
                    for j in range(JT):
                        for ot in range(2):
                            nc.tensor.matmul(
                                ps[ot][j], act_tile(k, j), w_t[:, kk, ot, :],
                                start=(k == 0), stop=(k == KT - 1),
                            )
                    if k == 10:
                        # bias: ones-row (K=1) x bias-row accumulate; grouped
                        # so the ones stationary loads once
                        for j in range(JT):
                            for ot in range(2):
                                nc.tensor.matmul(
                                    ps[ot][j], ones_sb[0:1, :],
                                    bias_sb[0:1, ot * 512 : (ot + 1) * 512],
                                    start=False, stop=False,
                                )
            # val = total row sum / (2*D)
            nc.vector.tensor_scalar_mul(val_sb, csum_sb, 1.0 / (2 * D))
            for j in range(JT):
                for ot in range(2):
                    epilogue(ps[ot][j], j, ot)

            # ---- pair 1: k-outer ----
            ps = [
                [
                    pspool.tile([128, 512], dt.float32, tag="ps", name=f"ps1_{ot}_{j}")
                    for j in range(JT)
                ]
                for ot in range(2)
            ]
            for kp in range(KT // 2):
                w_t = w_pair_tile(kp, 2)
                for kk in range(2):
                    k = 2 * kp + kk
                    for j in range(JT):
                        for ot in range(2):
                            nc.tensor.matmul(
                                ps[ot][j], act_tile(k, j), w_t[:, kk, ot, :],
                                start=(k == 0), stop=(k == KT - 1),
                            )
                    if k == 10:
                        for j in range(JT):
                            for ot in range(2):
                                nc.tensor.matmul(
                                    ps[ot][j], ones_sb[0:1, :],
                                    bias_sb[0:1, (2 + ot) * 512 : (3 + ot) * 512],
                                    start=False, stop=False,
                                )
            for j in range(JT):
                for ot in range(2):
                    epilogue(ps[ot][j], j, 2 + ot)
    nc.compile()
    return nc


def _np_mmdt():
    if MM_DTYPE == "float16":
        return np.float16
    if MM_DTYPE == "bfloat16":
        import ml_dtypes

        return np.dtype(ml_dtypes.bfloat16)
    return np.float32  # float32 / float32r


def _shard_inputs(x, W, b):
    ndt = _np_mmdt()
    x = np.ascontiguousarray(x, dtype=np.float32)
    W = np.asarray(W, dtype=np.float32)
    b = np.asarray(b, dtype=np.float32)
    wT_shards = [
        np.ascontiguousarray(W[c * O_LOC : (c + 1) * O_LOC, :].T).astype(ndt)
        for c in range(F_SHARDS)
    ]
    bias_shards = [
        b[c * O_LOC : (c + 1) * O_LOC].reshape(1, O_LOC).astype(ndt)
        for c in range(F_SHARDS)
    ]
    ones = np.ones((128, 128), dtype=ndt)
    in_maps = []
    for core in range(M_SHARDS * F_SHARDS):
        r, c = core % M_SHARDS, core // M_SHARDS
        b_sl = slice(r * B_LOC, (r + 1) * B_LOC)
        in_maps.append(
            dict(
                xaT=np.ascontiguousarray(x[b_sl, :, 0].T).astype(ndt),
                xn=x[b_sl].reshape(B_LOC, 2 * D).astype(ndt),
                wT=wT_shards[c],
                bias=bias_shards[c],
                ones=ones,
            )
        )
    return in_maps


def _gather(results):
    out = np.empty((B, D, 2), dtype=np.float32)
    for core, r in enumerate(results):
        m, c = core % M_SHARDS, core // M_SHARDS
        out[m * B_LOC : (m + 1) * B_LOC, c * O_LOC : (c + 1) * O_LOC, :] = r["out"]
    return out


def _run(x, W, b, trace=False, **spmd_kwargs):
    in_maps = _shard_inputs(x, W, b)
    nc = _build()
    res = run_bass_kernel_spmd(
        nc, in_maps, core_ids=list(range(8)), trace=trace, **spmd_kwargs
    )
    return _gather(res.results), res


def kernel(x, W, b):
    out, _ = _run(x, W, b, trace=False)
    return out


# revision 21
# speedup vs baseline: 1.0130x; 1.0130x over previous
"""TRN2 Bass kernel for nn_AVNNType1Linear.

Reference computation (B=2048, D_in=D_out=4096):
    act, carry = x[..., 0], x[..., 1]                  # x: [B, D_in, 2] f32
    act_out    = relu(act @ W.T + b)                   # [B, D_out]
    val        = 0.5*(mean(act, -1) + mean(carry, -1)) # [B]
    out        = stack([act_out, broadcast(val)], -1)  # [B, D_out, 2]

Distribution over 8 NeuronCores: 4-way data-parallel over batch x 2-way
tensor-parallel over output features (minimizes per-core HBM traffic:
xaT 8.4MB + xc 8.4MB + W.T-half 33.6MB + out 8.4MB ~= 59MB/core).

Per-core kernel: single-pass float32r matmul (full PE rate, ~2**-13
relative rounding vs ~2**-8 for bf16). The activator comes in host-
transposed ([D_in, B_loc]) so contraction sits on the partition dim with
clean DMAs; bias is folded into PSUM via a K=1 ones-row matmul; the
activator row-sums fall out of N=1 ones-column matmuls accumulated in
PSUM alongside the main GEMM; carry row-sums are a DVE free-dim reduce
of the naturally-laid-out carry channel. The [b, (o,ch)] interleaved
output tile is assembled in SBUF so the store DMA writes contiguous 4KB
rows.
"""

import os

import numpy as np

import concourse.mybir as mybir
import concourse.tile as tile
from concourse import bacc
from concourse.bass_utils import run_bass_kernel_spmd

B, D = 2048, 4096          # batch, D_in == D_out
M_SHARDS, F_SHARDS = 4, 2  # batch x feature grid over 8 cores
B_LOC = B // M_SHARDS      # 512 batch rows per core
O_LOC = D // F_SHARDS      # 2048 output features per core
KT = D // 128              # 32 contraction tiles
OT = O_LOC // 512          # 4 output tiles of 512
JT = B_LOC // 128          # 4 batch tiles of 128
KG = 8                     # activator SBUF tiles (groups of KT//KG k-tiles)
KPG = KT // KG


MM_DTYPE = os.environ.get("MM_DTYPE", "float16")


def _build():
    dt = mybir.dt
    mmdt = getattr(dt, MM_DTYPE)
    nc = bacc.Bacc("TRN2", target_bir_lowering=False, debug=False)
    xaT = nc.dram_tensor("xaT", [D, B_LOC], mmdt, kind="ExternalInput").ap()
    xn = nc.dram_tensor("xn", [B_LOC, 2 * D], mmdt, kind="ExternalInput").ap()
    wT = nc.dram_tensor("wT", [D, O_LOC], mmdt, kind="ExternalInput").ap()
    bias = nc.dram_tensor("bias", [1, O_LOC], mmdt, kind="ExternalInput").ap()
    ones = nc.dram_tensor("ones", [128, 128], mmdt, kind="ExternalInput").ap()
    out = nc.dram_tensor(
        "out", [B_LOC, O_LOC, 2], dt.float32, kind="ExternalOutput"
    ).ap()

    with tile.TileContext(nc) as tc:
        with (
            tc.tile_pool(name="persist", bufs=1) as persist,
            tc.tile_pool(name="wstream", bufs=6) as wpool,
            tc.tile_pool(name="xcpool", bufs=4) as xcpool,
            tc.tile_pool(name="opool", bufs=4) as opool,
            tc.tile_pool(name="small", bufs=1) as small,
            tc.tile_pool(name="ps", bufs=8, space="PSUM") as pspool,
        ):
            # --- persistent tiles (loads deferred off the critical path) --
            ones_sb = persist.tile([128, 128], mmdt)
            bias_sb = persist.tile([1, O_LOC], mmdt)

            def load_consts():
                nc.sync.dma_start(out=ones_sb, in_=ones)
                nc.sync.dma_start(out=bias_sb, in_=bias)

            # whole activator shard, [i%128, kt, b] layout, in KG chunks.
            # Only group 0 loads up front; later groups are emitted inside
            # the first k-loop so the w stream isn't queued behind them.
            act_g = [
                persist.tile([128, KPG, B_LOC], mmdt, tag=f"act{g}", name=f"act{g}")
                for g in range(KG)
            ]

            def load_act(g):
                nc.sync.dma_start(
                    out=act_g[g],
                    in_=xaT[g * KPG * 128 : (g + 1) * KPG * 128, :].rearrange(
                        "(kt p) b -> p kt b", p=128
                    ),
                )

            load_act(0)

            def act_tile(k, j):
                return act_g[k // KPG][:, k % KPG, j * 128 : (j + 1) * 128]

            # per-row total sums (both channels) -> val; the xn DMAs are
            # spread through the o==0 k-loop so they share bandwidth with
            # the w stream instead of ever queuing ahead of it
            csum_sb = small.tile([128, JT], dt.float32)
            val_sb = small.tile([128, JT], dt.float32)

            def row_sums(j):
                xn_t = xcpool.tile([128, 2 * D], mmdt, tag="xn", name=f"xn_{j}")
                nc.sync.dma_start(out=xn_t, in_=xn[j * 128 : (j + 1) * 128, :])
                nc.vector.reduce_sum(
                    csum_sb[:, j : j + 1], xn_t, axis=mybir.AxisListType.X
                )
            # o-pairs: all 8 PSUM banks live so each stationary act tile
            # feeds 2 consecutive matmuls (the weight-load cost amortizes).
            # Pair 0 runs k-outer (row sums/val finish early for its
            # epilogues); pair 1 runs j-outer over resident w tiles so the
            # epilogue + store of each batch tile streams out while later
            # batch tiles still compute (short kernel tail).

            def epilogue(ps_t, j, o):
                o_sl = slice(o * 512, (o + 1) * 512)
                out_t = opool.tile(
                    [128, 512, 2], dt.float32, tag="out", name=f"out_{o}_{j}"
                )
                nc.vector.tensor_scalar_max(out_t[:, :, 0], ps_t, 0.0)
                nc.vector.tensor_scalar(
                    out_t[:, :, 1], ps_t, 0.0, val_sb[:, j : j + 1],
                    op0=mybir.AluOpType.mult, op1=mybir.AluOpType.add,
                )
                eng = nc.sync if (j + o) % 2 == 0 else nc.gpsimd
                eng.dma_start(out=out[j * 128 : (j + 1) * 128, o_sl, :], in_=out_t)

            def w_pair_tile(kp, o0):
                w_t = wpool.tile(
                    [128, 2, 2, 512], mmdt, tag="wt", name=f"wt_{o0}_{kp}"
                )
                nc.sync.dma_start(
                    out=w_t,
                    in_=wT[
                        kp * 256 : (kp + 1) * 256, o0 * 512 : (o0 + 2) * 512
                    ].rearrange("(kt p) (ot n) -> p kt ot n", p=128, n=512),
                )
                return w_t

            # ---- pair 0: k-outer ----
            ps = [
                [
                    pspool.tile([128, 512], dt.float32, tag="ps", name=f"ps0_{ot}_{j}")
                    for j in range(JT)
                ]
                for ot in range(2)
            ]
            for kp in range(KT // 2):
                if 1 <= kp <= KG - 1:
                    load_act(kp)
                if kp == 1:
                    load_consts()
                if kp in (5, 7, 9, 11):
                    row_sums((kp - 5) // 2)
                w_t = w_pair_tile(kp, 0)
                for kk in range(2):
                    k = 2 * kp + kk
                    for j in range(JT):
                        for ot in range(2):
                            nc.tensor.matmul(
                                ps[ot][j], act_tile(k, j), w_t[:, kk, ot, :],
                                start=(k == 0), stop=(k == KT - 1),
                            )
                    if k == 10:
                        # bias: ones-row (K=1) x bias-row accumulate; grouped
                        # so the ones stationary loads once
                        for j in range(JT):
                            for ot in range(2):
                                nc.tensor.matmul(
                                    ps[ot][j], ones_sb[0:1, :],
                                    bias_sb[0:1, ot * 512 : (ot + 1) * 512],
                                    start=False, stop=False,
                                )
            # val = total row sum / (2*D)
            nc.vector.tensor_scalar_mul(val_sb, csum_sb, 1.0 / (2 * D))
            for j in range(JT):
                for ot in range(2):
                    epilogue(ps[ot][j], j, ot)

            # ---- pair 1: k-outer ----
            ps = [
                [
                    pspool.tile([128, 512], dt.float32, tag="ps", name=f"ps1_{ot}_{j}")
                    for j in range(JT)
                ]
                for ot in range(2)
            ]
            for kp in range(KT // 2):
                w_t = w_pair_tile(kp, 2)
                for kk in range(2):
                    k = 2 * kp + kk
                    for j in range(JT):
                        for ot in range(2):
                            nc.tensor.matmul(
                                ps[ot][j], act_tile(k, j), w_t[:, kk, ot, :],
                                start=(k == 0), stop=(k == KT - 1),
                            )
                    if k == 10:
                        for j in range(JT):
                            for ot in range(2):
                                nc.tensor.matmul(
                                    ps[ot][j], ones_sb[0:1, :],
                                    bias_sb[0:1, (2 + ot) * 512 : (3 + ot) * 512],
                                    start=False, stop=False,
                                )
            for j in range(JT):
                for ot in range(2):
                    epilogue(ps[ot][j], j, 2 + ot)
    nc.compile()
    return nc


def _np_mmdt():
    if MM_DTYPE == "float16":
        return np.float16
    if MM_DTYPE == "bfloat16":
        import ml_dtypes

        return np.dtype(ml_dtypes.bfloat16)
    return np.float32  # float32 / float32r


def _shard_inputs(x, W, b):
    ndt = _np_mmdt()
    x = np.ascontiguousarray(x, dtype=np.float32)
    W = np.asarray(W, dtype=np.float32)
    b = np.asarray(b, dtype=np.float32)
    wT_shards = [
        np.ascontiguousarray(W[c * O_LOC : (c + 1) * O_LOC, :].T).astype(ndt)
        for c in range(F_SHARDS)
    ]
    bias_shards = [
        b[c * O_LOC : (c + 1) * O_LOC].reshape(1, O_LOC).astype(ndt)
        for c in range(F_SHARDS)
    ]
    ones = np.ones((128, 128), dtype=ndt)
    in_maps = []
    for core in range(M_SHARDS * F_SHARDS):
        r, c = core % M_SHARDS, core // M_SHARDS
        b_sl = slice(r * B_LOC, (r + 1) * B_LOC)
        in_maps.append(
            dict(
                xaT=np.ascontiguousarray(x[b_sl, :, 0].T).astype(ndt),
                xn=x[b_sl].reshape(B_LOC, 2 * D).astype(ndt),
                wT=wT_shards[c],
                bias=bias_shards[c],
                ones=ones,
            )
        )
    return in_maps


def _gather(results):
    out = np.empty((B, D, 2), dtype=np.float32)
    for core, r in enumerate(results):
        m, c = core % M_SHARDS, core // M_SHARDS
        out[m * B_LOC : (m + 1) * B_LOC, c * O_LOC : (c + 1) * O_LOC, :] = r["out"]
    return out


def _run(x, W, b, trace=False, **spmd_kwargs):
    in_maps = _shard_inputs(x, W, b)
    nc = _build()
    res = run_bass_kernel_spmd(
        nc, in_maps, core_ids=list(range(8)), trace=trace, **spmd_kwargs
    )
    return _gather(res.results), res


def kernel(x, W, b):
    out, _ = _run(x, W, b, trace=False)
    return out
